# revision 13
# baseline (speedup 1.0000x reference)
"""Trainium2 Bass kernel for DirectTargetLoss.

Computes, from sparse_rep [256, 128000] f32 and target_ids [256, 16] i64:
  target_loss   = -mean(log(gather(sparse_rep, target_ids) + 1e-8))
  margin_loss   = mean(relu(1 - gather(sparse_rep, target_ids)))
  negative_loss = mean(top_k(sparse_rep with target cols masked to -1e30, 100))

Sharding: data-parallel over the batch axis across 8 NeuronCores
(32 rows/core).  Per core:
  - the [32, 128000] shard is streamed into SBUF as 8 tiles of
    [128, 4000] f32 (each tile = 4 rows x 2 column-halves of 64000;
    each half spread over 16 partitions),
  - the gpsimd top-256 instruction runs per tile (8 "tokens" = 8
    half-rows; exact, sorted top-256 values + within-half indices),
  - per row, the top-128 of each half (sorted tails) are concatenated
    to a 256-candidate tile; candidates whose vocab index matches one
    of the row's <=16 target ids are zeroed (so the masked top-100 of
    the full row is exactly the top-100 of the surviving candidates),
  - 13 rounds of max8 + match_replace extract the top 100 exactly,
  - the 16 target activations per row are gathered with an indirect
    DMA and pushed through Ln / Relu activations with accumulation,
  - a ones-vector matmul reduces the three per-row partials to [1, 3].
Host sums the 8 per-core [1,3] partials and normalizes.
"""

import numpy as np

B = 256
V = 128000
T = 16
TOP_K = 100
EPS = 1e-8
N_CORES = 8
BL = B // N_CORES          # 32 rows per core
RPT = 4                    # rows per topk call (2 halves each -> 8 tokens)
NT = BL // RPT             # 8 tiles per core
HALF = V // 2              # 64000
SEG = 16                   # partitions per half-row
F = HALF // SEG            # 4000 free elems per partition
KC = 256                   # topk k (per half-row)
TAIL = 128                 # kept tail per half
NC_CAND = 2 * TAIL         # 256 candidates per row

_CACHE = {}


def _build_nc():
    from contextlib import ExitStack

    import concourse.bass as bass
    import concourse.tile as tile
    from concourse import bacc, mybir

    f32 = mybir.dt.float32
    i32 = mybir.dt.int32
    u32 = mybir.dt.uint32
    AF = mybir.ActivationFunctionType
    OP = mybir.AluOpType

    nc = bacc.Bacc("TRN2", target_bir_lowering=False, debug=False)

    sp = nc.dram_tensor("sp", [BL, V], f32, kind="ExternalInput")
    off = nc.dram_tensor("off", [BL, T], i32, kind="ExternalInput")
    idsf = nc.dram_tensor("idsf", [BL, T], f32, kind="ExternalInput")
    out3 = nc.dram_tensor("out3", [1, 3], f32, kind="ExternalOutput")

    with tile.TileContext(nc) as tc, ExitStack() as ctx:
        cand_pool = ctx.enter_context(tc.tile_pool(name="cand", bufs=1))
        small_pool = ctx.enter_context(tc.tile_pool(name="small", bufs=1))
        psum_pool = ctx.enter_context(tc.tile_pool(name="psum", bufs=1, space="PSUM"))

        valsF = cand_pool.tile([BL, 2 * KC], f32, tag="valsF")
        idxF = cand_pool.tile([BL, 2 * KC], u32, tag="idxF")
        valst = cand_pool.tile([BL, NC_CAND], f32, tag="valst")
        idxt = cand_pool.tile([BL, NC_CAND], f32, tag="idxt")
        km = cand_pool.tile([BL, NC_CAND], f32, tag="km")
        cur0 = cand_pool.tile([BL, NC_CAND], f32, tag="cur0")
        cur1 = cand_pool.tile([BL, NC_CAND], f32, tag="cur1")

        ids_sb = small_pool.tile([BL, T], f32, tag="ids_sb")
        off_sb = small_pool.tile([BL, T], i32, tag="off_sb")
        tgt = small_pool.tile([BL, T], f32, tag="tgt")
        lnout = small_pool.tile([BL, T], f32, tag="lnout")
        mgout = small_pool.tile([BL, T], f32, tag="mgout")
        eps_t = small_pool.tile([BL, 1], f32, tag="eps_t")
        mx = small_pool.tile([BL, 8], f32, tag="mx")
        mxacc = small_pool.tile([BL, 8], f32, tag="mxacc")
        negrow = small_pool.tile([BL, 1], f32, tag="negrow")
        lnrow = small_pool.tile([BL, 1], f32, tag="lnrow")
        mgrow = small_pool.tile([BL, 1], f32, tag="mgrow")
        stacked = small_pool.tile([BL, 3], f32, tag="stacked")
        ones = small_pool.tile([BL, 1], f32, tag="ones")
        out_sb = small_pool.tile([1, 3], f32, tag="out_sb")

        # --- small input loads + target gather (independent of big loads) ---
        nc.sync.dma_start(ids_sb[:], idsf[:, :])
        nc.sync.dma_start(off_sb[:], off[:, :])
        sp_flat = sp[:, :].rearrange("b (v one) -> (b v) one", one=1)
        nc.gpsimd.indirect_dma_start(
            out=tgt[:],
            out_offset=None,
            in_=sp_flat,
            in_offset=bass.IndirectOffsetOnAxis(ap=off_sb[:, :], axis=0),
        )

        # target_loss partial: sum(log(tgt + eps)) per row
        nc.vector.memset(eps_t[:], EPS)
        nc.scalar.activation(
            lnout[:], tgt[:], AF.Ln,
            bias=eps_t[:, 0:1], scale=1.0, accum_out=lnrow[:],
        )
        # margin_loss partial: sum(relu(1 - tgt)) per row
        nc.scalar.activation(
            mgout[:], tgt[:], AF.Relu,
            bias=1.0, scale=-1.0, accum_out=mgrow[:],
        )

        # --- big loads + per-half top-256 ---
        # tile c covers rows 4c..4c+3; token (j,h) = row 4c+j, col half h
        # sits on partitions 16*(2j+h) .. +15, 4000 contiguous elems each.
        for c in range(NT):
            data = nc.alloc_sbuf_tensor(f"data{c}", [128, F], f32).ap()
            src = sp[RPT * c:RPT * (c + 1), :].rearrange(
                "j (h s f) -> (j h s) f", h=2, s=SEG
            )
            nc.sync.dma_start(data[:], src)

            tout = nc.alloc_sbuf_tensor(f"tout{c}", [128, 32], u32).ap()
            nc.gpsimd.topk(tout[:], data[:], tokens=8, vocab_size=HALF, k=KC)

            # all 256 sorted-asc values per half -> row-major [4 rows, 2*256]
            # (row r cols = [h0 asc 0..255 | h1 asc 0..255])
            dst_v = valsF[RPT * c:RPT * (c + 1), :].rearrange(
                "j (h s f) -> j h s f", h=2, f=16
            )
            nc.sync.dma_start(dst_v, tout[:, 0:16].bitcast(f32))
            dst_i = idxF[RPT * c:RPT * (c + 1), :].rearrange(
                "j (h s f) -> j h s f", h=2, f=16
            )
            nc.sync.dma_start(dst_i, tout[:, 16:32])

        # candidate tails: top-128 of each half (ascending, cols 128..256 and
        # 384..512 of the staging tiles); indices cast u32->f32, +HALF for
        # the second half so they are vocab ids
        nc.vector.tensor_copy(valst[:, 0:TAIL], valsF[:, KC - TAIL:KC])
        nc.vector.tensor_copy(valst[:, TAIL:NC_CAND], valsF[:, 2 * KC - TAIL:2 * KC])
        nc.vector.tensor_copy(idxt[:, 0:TAIL], idxF[:, KC - TAIL:KC])
        nc.vector.tensor_scalar_add(
            idxt[:, TAIL:NC_CAND], idxF[:, 2 * KC - TAIL:2 * KC], float(HALF)
        )

        # keep-mask: 1 where candidate vocab id is NOT any of the row's targets
        nc.vector.memset(km[:], 1.0)
        for t in range(T):
            nc.vector.scalar_tensor_tensor(
                out=km[:], in0=idxt[:], scalar=ids_sb[:, t:t + 1], in1=km[:],
                op0=OP.not_equal, op1=OP.mult,
            )

        # masked candidates (>=240 real values, all > 0; zeros elsewhere)
        nc.vector.tensor_tensor(cur0[:], valst[:], km[:], op=OP.mult)

        # exact top-100: 12 rounds of (max8, accumulate, zap) + 4 of round 13
        nc.vector.memset(mxacc[:], 0.0)
        bufs = [cur0, cur1]
        for r in range(13):
            cur = bufs[r % 2]
            nxt = bufs[(r + 1) % 2]
            nc.vector.max(mx[:], cur[:])
            if r < 12:
                nc.vector.tensor_add(mxacc[:], mxacc[:], mx[:])
                nc.vector.match_replace(
                    out=nxt[:], in_to_replace=mx[:], in_values=cur[:],
                    imm_value=0.0,
                )
            else:
                # ranks 97..104 in descending order; keep 97..100
                nc.vector.tensor_add(mxacc[:, 0:4], mxacc[:, 0:4], mx[:, 0:4])
        nc.vector.tensor_reduce(
            negrow[:], mxacc[:], axis=mybir.AxisListType.X, op=OP.add,
        )

        # stack [BL, 3] = [lnrow, mgrow, negrow], reduce over rows via matmul
        nc.vector.tensor_copy(stacked[:, 0:1], lnrow[:])
        nc.vector.tensor_copy(stacked[:, 1:2], mgrow[:])
        nc.vector.tensor_copy(stacked[:, 2:3], negrow[:])
        nc.vector.memset(ones[:], 1.0)
        acc = psum_pool.tile([1, 3], f32, tag="acc")
        nc.tensor.matmul(acc[:], lhsT=ones[:], rhs=stacked[:], start=True, stop=True)
        nc.vector.tensor_copy(out_sb[:], acc[:])
        nc.sync.dma_start(out3[:, :], out_sb[:])

    nc.compile()
    return nc


def _get_nc():
    if "nc" not in _CACHE:
        _CACHE["nc"] = _build_nc()
    return _CACHE["nc"]


def make_in_maps(sparse_rep, target_ids):
    sp = np.ascontiguousarray(np.asarray(sparse_rep), dtype=np.float32)
    ids = np.asarray(target_ids)
    assert sp.shape == (B, V) and ids.shape == (B, T)
    in_maps = []
    rowoff = np.arange(BL, dtype=np.int64)[:, None] * V
    for i in range(N_CORES):
        rows = slice(BL * i, BL * (i + 1))
        idl = ids[rows].astype(np.int64)
        in_maps.append({
            "sp": sp[rows],
            "off": (rowoff + idl).astype(np.int32),
            "idsf": idl.astype(np.float32),
        })
    return in_maps


def combine(parts):
    """parts: list of 8 [1,3] arrays -> (target_loss, margin_loss, negative_loss)"""
    acc = np.zeros(3, np.float64)
    for p in parts:
        acc += np.asarray(p, dtype=np.float64).reshape(3)
    target_loss = np.float32(-(acc[0] / (B * T)))
    margin_loss = np.float32(acc[1] / (B * T))
    negative_loss = np.float32(acc[2] / (B * TOP_K))
    return (target_loss, margin_loss, negative_loss)


def kernel(sparse_rep, target_ids):
    from concourse.bass_utils import run_bass_kernel_spmd

    nc = _get_nc()
    in_maps = make_in_maps(sparse_rep, target_ids)
    res = run_bass_kernel_spmd(nc, in_maps, list(range(N_CORES))).results
    return combine([r["out3"] for r in res])


# revision 14
# speedup vs baseline: 1.0110x; 1.0110x over previous
"""Trainium2 Bass kernel for DirectTargetLoss.

Computes, from sparse_rep [256, 128000] f32 and target_ids [256, 16] i64:
  target_loss   = -mean(log(gather(sparse_rep, target_ids) + 1e-8))
  margin_loss   = mean(relu(1 - gather(sparse_rep, target_ids)))
  negative_loss = mean(top_k(sparse_rep with target cols masked to -1e30, 100))

Sharding: data-parallel over the batch axis across 8 NeuronCores
(32 rows/core).  Per core:
  - the [32, 128000] shard is streamed into SBUF as 8 tiles of
    [128, 4000] f32 (each tile = 4 rows x 2 column-halves of 64000;
    each half spread over 16 partitions),
  - the gpsimd top-256 instruction runs per tile (8 "tokens" = 8
    half-rows; exact, sorted top-256 values + within-half indices),
  - per row, the top-128 of each half (sorted tails) are concatenated
    to a 256-candidate tile; candidates whose vocab index matches one
    of the row's <=16 target ids are zeroed (so the masked top-100 of
    the full row is exactly the top-100 of the surviving candidates),
  - 13 rounds of max8 + match_replace extract the top 100 exactly,
  - the 16 target activations per row are gathered with an indirect
    DMA and pushed through Ln / Relu activations with accumulation,
  - a ones-vector matmul reduces the three per-row partials to [1, 3].
Host sums the 8 per-core [1,3] partials and normalizes.
"""

import numpy as np

B = 256
V = 128000
T = 16
TOP_K = 100
EPS = 1e-8
N_CORES = 8
BL = B // N_CORES          # 32 rows per core
RPT = 4                    # rows per topk call (2 halves each -> 8 tokens)
NT = BL // RPT             # 8 tiles per core
HALF = V // 2              # 64000
SEG = 16                   # partitions per half-row
F = HALF // SEG            # 4000 free elems per partition
KC = 256                   # topk k (per half-row)
TAIL = 128                 # kept tail per half
NC_CAND = 2 * TAIL         # 256 candidates per row

_CACHE = {}


def _build_nc():
    from contextlib import ExitStack

    import concourse.bass as bass
    import concourse.tile as tile
    from concourse import bacc, mybir

    f32 = mybir.dt.float32
    i32 = mybir.dt.int32
    u32 = mybir.dt.uint32
    AF = mybir.ActivationFunctionType
    OP = mybir.AluOpType

    nc = bacc.Bacc("TRN2", target_bir_lowering=False, debug=False)

    sp = nc.dram_tensor("sp", [BL, V], f32, kind="ExternalInput")
    off = nc.dram_tensor("off", [BL, T], i32, kind="ExternalInput")
    idsf = nc.dram_tensor("idsf", [BL, T], f32, kind="ExternalInput")
    out3 = nc.dram_tensor("out3", [1, 3], f32, kind="ExternalOutput")

    with tile.TileContext(nc) as tc, ExitStack() as ctx:
        cand_pool = ctx.enter_context(tc.tile_pool(name="cand", bufs=1))
        small_pool = ctx.enter_context(tc.tile_pool(name="small", bufs=1))
        psum_pool = ctx.enter_context(tc.tile_pool(name="psum", bufs=1, space="PSUM"))

        valsF = cand_pool.tile([BL, 2 * KC], f32, tag="valsF")
        idxF = cand_pool.tile([BL, 2 * KC], u32, tag="idxF")
        valst = cand_pool.tile([BL, NC_CAND], f32, tag="valst")
        idxt = cand_pool.tile([BL, NC_CAND], f32, tag="idxt")
        km = cand_pool.tile([BL, NC_CAND], f32, tag="km")
        cur0 = cand_pool.tile([BL, NC_CAND], f32, tag="cur0")
        cur1 = cand_pool.tile([BL, NC_CAND], f32, tag="cur1")

        ids_sb = small_pool.tile([BL, T], f32, tag="ids_sb")
        off_sb = small_pool.tile([BL, T], i32, tag="off_sb")
        tgt = small_pool.tile([BL, T], f32, tag="tgt")
        lnout = small_pool.tile([BL, T], f32, tag="lnout")
        mgout = small_pool.tile([BL, T], f32, tag="mgout")
        eps_t = small_pool.tile([BL, 1], f32, tag="eps_t")
        mx = small_pool.tile([BL, 8], f32, tag="mx")
        mxacc = small_pool.tile([BL, 8], f32, tag="mxacc")
        negrow = small_pool.tile([BL, 1], f32, tag="negrow")
        lnrow = small_pool.tile([BL, 1], f32, tag="lnrow")
        mgrow = small_pool.tile([BL, 1], f32, tag="mgrow")
        stacked = small_pool.tile([BL, 3], f32, tag="stacked")
        ones = small_pool.tile([BL, 1], f32, tag="ones")
        out_sb = small_pool.tile([1, 3], f32, tag="out_sb")

        # --- small input loads + target gather (independent of big loads) ---
        nc.sync.dma_start(ids_sb[:], idsf[:, :])
        nc.sync.dma_start(off_sb[:], off[:, :])
        sp_flat = sp[:, :].rearrange("b (v one) -> (b v) one", one=1)
        # HW DGE uses one offset per out partition-row and fetches
        # out-free-size consecutive elements, so gather column-by-column.
        for t in range(T):
            nc.gpsimd.indirect_dma_start(
                out=tgt[:, t:t + 1],
                out_offset=None,
                in_=sp_flat,
                in_offset=bass.IndirectOffsetOnAxis(ap=off_sb[:, t:t + 1], axis=0),
            )

        # target_loss partial: sum(log(tgt + eps)) per row
        nc.vector.memset(eps_t[:], EPS)
        nc.scalar.activation(
            lnout[:], tgt[:], AF.Ln,
            bias=eps_t[:, 0:1], scale=1.0, accum_out=lnrow[:],
        )
        # margin_loss partial: sum(relu(1 - tgt)) per row
        nc.scalar.activation(
            mgout[:], tgt[:], AF.Relu,
            bias=1.0, scale=-1.0, accum_out=mgrow[:],
        )

        # --- big loads + per-half top-256 ---
        # tile c covers rows 4c..4c+3; token (j,h) = row 4c+j, col half h
        # sits on partitions 16*(2j+h) .. +15, 4000 contiguous elems each.
        for c in range(NT):
            data = nc.alloc_sbuf_tensor(f"data{c}", [128, F], f32).ap()
            src = sp[RPT * c:RPT * (c + 1), :].rearrange(
                "j (h s f) -> (j h s) f", h=2, s=SEG
            )
            nc.sync.dma_start(data[:], src)

            tout = nc.alloc_sbuf_tensor(f"tout{c}", [128, 32], u32).ap()
            nc.gpsimd.topk(tout[:], data[:], tokens=8, vocab_size=HALF, k=KC)

            # all 256 sorted-asc values per half -> row-major [4 rows, 2*256]
            # (row r cols = [h0 asc 0..255 | h1 asc 0..255])
            dst_v = valsF[RPT * c:RPT * (c + 1), :].rearrange(
                "j (h s f) -> j h s f", h=2, f=16
            )
            nc.sync.dma_start(dst_v, tout[:, 0:16].bitcast(f32))
            dst_i = idxF[RPT * c:RPT * (c + 1), :].rearrange(
                "j (h s f) -> j h s f", h=2, f=16
            )
            nc.sync.dma_start(dst_i, tout[:, 16:32])

        # candidate tails: top-128 of each half (ascending, cols 128..256 and
        # 384..512 of the staging tiles); indices cast u32->f32, +HALF for
        # the second half so they are vocab ids
        nc.vector.tensor_copy(valst[:, 0:TAIL], valsF[:, KC - TAIL:KC])
        nc.vector.tensor_copy(valst[:, TAIL:NC_CAND], valsF[:, 2 * KC - TAIL:2 * KC])
        nc.vector.tensor_copy(idxt[:, 0:TAIL], idxF[:, KC - TAIL:KC])
        nc.vector.tensor_scalar_add(
            idxt[:, TAIL:NC_CAND], idxF[:, 2 * KC - TAIL:2 * KC], float(HALF)
        )

        # keep-mask: 1 where candidate vocab id is NOT any of the row's targets
        nc.vector.memset(km[:], 1.0)
        for t in range(T):
            nc.vector.scalar_tensor_tensor(
                out=km[:], in0=idxt[:], scalar=ids_sb[:, t:t + 1], in1=km[:],
                op0=OP.not_equal, op1=OP.mult,
            )

        # masked candidates (>=240 real values, all > 0; zeros elsewhere)
        nc.vector.tensor_tensor(cur0[:], valst[:], km[:], op=OP.mult)

        # exact top-100: 12 rounds of (max8, accumulate, zap) + 4 of round 13
        nc.vector.memset(mxacc[:], 0.0)
        bufs = [cur0, cur1]
        for r in range(13):
            cur = bufs[r % 2]
            nxt = bufs[(r + 1) % 2]
            nc.vector.max(mx[:], cur[:])
            if r < 12:
                nc.vector.tensor_add(mxacc[:], mxacc[:], mx[:])
                nc.vector.match_replace(
                    out=nxt[:], in_to_replace=mx[:], in_values=cur[:],
                    imm_value=0.0,
                )
            else:
                # ranks 97..104 in descending order; keep 97..100
                nc.vector.tensor_add(mxacc[:, 0:4], mxacc[:, 0:4], mx[:, 0:4])
        nc.vector.tensor_reduce(
            negrow[:], mxacc[:], axis=mybir.AxisListType.X, op=OP.add,
        )

        # stack [BL, 3] = [lnrow, mgrow, negrow], reduce over rows via matmul
        nc.vector.tensor_copy(stacked[:, 0:1], lnrow[:])
        nc.vector.tensor_copy(stacked[:, 1:2], mgrow[:])
        nc.vector.tensor_copy(stacked[:, 2:3], negrow[:])
        nc.vector.memset(ones[:], 1.0)
        acc = psum_pool.tile([1, 3], f32, tag="acc")
        nc.tensor.matmul(acc[:], lhsT=ones[:], rhs=stacked[:], start=True, stop=True)
        nc.vector.tensor_copy(out_sb[:], acc[:])
        nc.sync.dma_start(out3[:, :], out_sb[:])

    nc.compile()
    return nc


def _get_nc():
    if "nc" not in _CACHE:
        _CACHE["nc"] = _build_nc()
    return _CACHE["nc"]


def make_in_maps(sparse_rep, target_ids):
    sp = np.ascontiguousarray(np.asarray(sparse_rep), dtype=np.float32)
    ids = np.asarray(target_ids)
    assert sp.shape == (B, V) and ids.shape == (B, T)
    in_maps = []
    rowoff = np.arange(BL, dtype=np.int64)[:, None] * V
    for i in range(N_CORES):
        rows = slice(BL * i, BL * (i + 1))
        idl = ids[rows].astype(np.int64)
        in_maps.append({
            "sp": sp[rows],
            "off": (rowoff + idl).astype(np.int32),
            "idsf": idl.astype(np.float32),
        })
    return in_maps


def combine(parts):
    """parts: list of 8 [1,3] arrays -> (target_loss, margin_loss, negative_loss)"""
    acc = np.zeros(3, np.float64)
    for p in parts:
        acc += np.asarray(p, dtype=np.float64).reshape(3)
    target_loss = np.float32(-(acc[0] / (B * T)))
    margin_loss = np.float32(acc[1] / (B * T))
    negative_loss = np.float32(acc[2] / (B * TOP_K))
    return (target_loss, margin_loss, negative_loss)


def kernel(sparse_rep, target_ids):
    from concourse.bass_utils import run_bass_kernel_spmd

    nc = _get_nc()
    in_maps = make_in_maps(sparse_rep, target_ids)
    res = run_bass_kernel_spmd(nc, in_maps, list(range(N_CORES))).results
    return combine([r["out3"] for r in res])
